# revision 5
# baseline (speedup 1.0000x reference)
"""Trainium2 Bass kernel for nn_CLPredictor (dense pairwise MLP).

Computation (see reference):
  per-segment MLP  : hat_p = softmax(MLP3(agg_r)), loss1 = -mean(logp[y])
  pairwise MLP     : for all (i, j) pairs per batch,
                     hq  = relu(a_i + b_j + bs1)         (a/b = agg_r @ Ws1 halves)
                     hq2 = relu(hq @ Ws2 + bs2)
                     delta = hq2 . (Ws3[:,1]-Ws3[:,0]) + (bs3[1]-bs3[0])
                     q1 = sigmoid(delta), loss2 = mean(softplus(delta) - same*delta)
  tilde_p[i]       = normalize(sum_j q1[i,j] * hat_p[j])

Sharding: 8 cores, each owns 64 consecutive (b, i) rows of the B*N x N
pairwise grid (cores 0-3 -> batch 0, cores 4-7 -> batch 1); params
replicated; per-segment MLP computed per-batch (replicated within the 4
cores of a batch; hat_p halves taken from cores 0 and 4).

All activations are kept transposed ([feature, segment]) so the pairwise
broadcast add runs as one ACT/DVE op per tile with a per-partition bias.
"""

import numpy as np

import concourse.bass as bass
import concourse.mybir as mybir
import concourse.tile as tile
from concourse.bass_utils import run_bass_kernel_spmd
from concourse.masks import make_identity

# ---------------------------------------------------------------- constants
B, N, D, H, C = 2, 256, 512, 1024, 6
H2 = H // 2          # 512
BN = B * N           # 512
NCORES = 8
RPC = N // (NCORES // B)   # 64 i-rows per core
NIT = RPC // 2             # 32 main-loop iterations (2 i's each)

f32 = mybir.dt.float32
f32r = mybir.dt.float32r
bf16 = mybir.dt.bfloat16
AF = mybir.ActivationFunctionType
ALU = mybir.AluOpType
AX = mybir.AxisListType

# Matmul mode for the heavy pairwise path and the prologue matmuls.
#   "f32r": fp32 data, fast PE mode (1 cyc/row when N>=256)
#   "bf16": bf16 operands (half SBUF traffic), fp32 PSUM accumulation
#   "f32" : plain fp32 (4 cyc/row)
MM_MODE = "f32r"

# dtype used for every tensor consumed by the fast matmuls (walrus requires
# producers of fp32r matmul operands to emit rounded fp32r, so the tiles and
# DRAM tensors themselves carry the dtype; numpy side stays float32).
_MMDT = {"f32r": f32r, "bf16": bf16, "f32": f32}[MM_MODE]


def _cast(ap):
    return ap


# ------------------------------------------------- walrus wait workaround
# This container's walrus accepts at most ONE sync wait per instruction
# (setupSyncWait: "Too many sync wait commands"), while Tile freely
# attaches several.  Two patches:
#  1. after wait assignment, split any instruction with >1 waits by
#     hoisting the extras onto same-engine InstNoOp carriers emitted just
#     before it (engine program order then gates the instruction);
#  2. the Tile exit Drain (which waits on the whole global clock) gets the
#     same treatment via nop carriers on the sync engine.
_MAX_WAITS = 1


def _patch_tile_waits():
    import bass_rust as _br
    from concourse.vector_clock import ScopedClock

    orig_lower = tile.TileContext._lower_ordered_insts

    def _split_lower(self, ordered):
        nc = self.nc
        for bb_name, insts in ordered.items():
            new_insts = []
            for inst in insts:
                si = getattr(inst, "sync_info", None)
                waits = list(si.on_wait) if si is not None and si.on_wait else []
                if len(waits) > _MAX_WAITS and not isinstance(
                    inst, tile.BassTileLoopBlock
                ):
                    keep = waits[-_MAX_WAITS:]
                    extra = waits[:-_MAX_WAITS]
                    for w in extra:
                        nop = mybir.InstNoOp(
                            name=f"I-{nc.next_id()}",
                            sync_info=mybir.SyncInfo(on_wait=[w], on_update=[]),
                            bass_nofuse=True,
                            engine=inst.engine,
                        )
                        new_insts.append(nop)
                    si.on_wait = keep
                new_insts.append(inst)
            insts[:] = new_insts
        return orig_lower(self, ordered)

    tile.TileContext._lower_ordered_insts = _split_lower

    def _split_drain_and_barrier(self, tick_clock, wait_clock):
        nc = self.nc
        carrier = nc.sync.nop()
        wait_clock.add_sem_waits(
            carrier.ins, ScopedClock({None: tick_clock.global_clock})
        )
        si = carrier.ins.sync_info
        waits = list(si.on_wait) if si and si.on_wait else []
        if len(waits) > 1:
            si.on_wait = waits[:1]
            for w in waits[1:]:
                extra = nc.sync.nop()
                extra.ins.sync_info = _br.SyncInfo(on_wait=[w], on_update=[])

        nc.sync.drain()
        nc.all_engine_barrier()
        assert self.sems is not None
        popped = nc._tile_sem_poison_stack.pop()
        assert popped is self._sem_poison
        nc.clear_and_free_semaphores(list(self.sems.allocated().values()))
        nc.all_engine_barrier()

    tile.TileContext._drain_and_barrier = _split_drain_and_barrier


_patch_tile_waits()


# ------------------------------------------------------------ device program
def build_program():
    nc = bass.Bass("TRN2", target_bir_lowering=False, debug=False)

    def din(name, shape, dt=f32):
        return nc.dram_tensor(name, shape, dt, kind="ExternalInput")

    # per-core inputs (b = core's batch, i0 = core's first local row)
    xTb = din("xTb", [D, N], _MMDT)   # agg_r[b].T
    xTo = din("xTo", [D, RPC], _MMDT) # agg_r[b][i0:i0+RPC].T
    yfb = din("yfb", [N, C])          # one-hot y[b] as f32
    yft_e = din("yft_e", [C, NIT])    # yfb.T cols i0, i0+2, ...
    yft_o = din("yft_o", [C, NIT])    # yfb.T cols i0+1, i0+3, ...
    yft_b = din("yft_b", [C, N])      # yfb.T
    # replicated params
    W1 = din("W1", [D, H], _MMDT)
    W2 = din("W2", [H, H2], _MMDT)
    W3 = din("W3", [H2, C], _MMDT)
    b1c = din("b1c", [128, H // 128])
    b2c = din("b2c", [128, H2 // 128])
    b3bc = din("b3bc", [128, C])
    Ws1a = din("Ws1a", [D, H], _MMDT)
    Ws1b = din("Ws1b", [D, H], _MMDT)
    bs1c = din("bs1c", [128, H // 128])
    Ws2m = din("Ws2m", [H, H2], _MMDT)
    bs2c = din("bs2c", [128, H2 // 128])
    dvecc = din("dvecc", [128, H2 // 128], _MMDT)
    bdc = din("bdc", [NIT, 1])

    hatp_o = nc.dram_tensor("hatp", [N, C], f32, kind="ExternalOutput")
    tilde_o = nc.dram_tensor("tilde", [RPC, C], f32, kind="ExternalOutput")
    lparts_o = nc.dram_tensor("lparts", [128, 6], f32, kind="ExternalOutput")

    KD = D // 128    # 4 contraction chunks over D
    KH = H // 128    # 8 chunks over H
    KH2 = H2 // 128  # 4 chunks over H2

    with tile.TileContext(nc) as tc:
        # ---------------- persistent tiles (live across the whole kernel)
        with tc.tile_pool(name="persist", bufs=1) as pp:
            bT = pp.tile([128, KH, N], f32)          # b-side, batch cols
            aprime = pp.tile([128, KH, RPC], f32)    # a-side + bs1, our cols
            Ws2_sb = pp.tile([128, KH, H2], _MMDT)
            dvec_sb = pp.tile([128, KH2], _MMDT)
            bs2c_sb = pp.tile([128, KH2], f32)
            bdc_sb = pp.tile([NIT, 1], f32)
            hatp_sb = pp.tile([128, N // 128, C], f32)
            dstack = pp.tile([NIT, 2 * N], f32)      # delta rows
            sstack = pp.tile([NIT, 2 * N], f32)      # same rows
            lparts = pp.tile([128, 6], f32)
            ident = pp.tile([NIT, NIT], f32)

            make_identity(nc, ident[:])
            nc.vector.memset(lparts[:], 0.0)

            # ---------------- phase 1: per-seg MLP + a/b precompute
            with tc.tile_pool(name="ph1", bufs=1) as p1, \
                 tc.tile_pool(name="ph1ps", bufs=2, space="PSUM") as p1ps, \
                 tc.tile_pool(name="ph1ps2", bufs=2, space="PSUM") as p1ps2:
                xTb_sb = p1.tile([128, KD, N], _MMDT)
                xTo_sb = p1.tile([128, KD, RPC], _MMDT)
                W1_sb = p1.tile([128, KD, H], _MMDT)
                W2_sb = p1.tile([128, KH, H2], _MMDT)
                W3_sb = p1.tile([128, KH2, C], _MMDT)
                Ws1a_sb = p1.tile([128, KD, H], _MMDT)
                Ws1b_sb = p1.tile([128, KD, H], _MMDT)
                b1c_sb = p1.tile([128, KH], f32)
                b2c_sb = p1.tile([128, KH2], f32)
                b3bc_sb = p1.tile([128, C], f32)
                bs1c_sb = p1.tile([128, KH], f32)
                yfb_sb = p1.tile([128, N // 128, C], f32)
                yfte_sb = p1.tile([C, NIT], f32)
                yfto_sb = p1.tile([C, NIT], f32)
                yftb_sb = p1.tile([C, N], f32)
                h1T = p1.tile([128, KH, N], _MMDT)
                h2T = p1.tile([128, KH2, N], _MMDT)

                nc.sync.dma_start(xTb_sb[:], xTb.rearrange("(kc p) m -> p kc m", p=128))
                nc.sync.dma_start(xTo_sb[:], xTo.rearrange("(kc p) m -> p kc m", p=128))
                nc.sync.dma_start(W1_sb[:], W1.rearrange("(kc p) m -> p kc m", p=128))
                nc.sync.dma_start(W2_sb[:], W2.rearrange("(kc p) m -> p kc m", p=128))
                nc.sync.dma_start(W3_sb[:], W3.rearrange("(kc p) m -> p kc m", p=128))
                nc.sync.dma_start(Ws1a_sb[:], Ws1a.rearrange("(kc p) m -> p kc m", p=128))
                nc.sync.dma_start(Ws1b_sb[:], Ws1b.rearrange("(kc p) m -> p kc m", p=128))
                nc.sync.dma_start(b1c_sb[:], b1c[:])
                nc.sync.dma_start(b2c_sb[:], b2c[:])
                nc.sync.dma_start(b3bc_sb[:], b3bc[:])
                nc.sync.dma_start(bs1c_sb[:], bs1c[:])
                nc.sync.dma_start(yfb_sb[:], yfb.rearrange("(mt p) c -> p mt c", p=128))
                nc.sync.dma_start(yfte_sb[:], yft_e[:])
                nc.sync.dma_start(yfto_sb[:], yft_o[:])
                nc.sync.dma_start(yftb_sb[:], yft_b[:])

                # persist-pool loads issued after the phase-1 weights so the
                # first h1 matmuls aren't stuck behind the 2MB Ws2 transfer
                nc.sync.dma_start(Ws2_sb[:], Ws2m.rearrange("(kc p) m -> p kc m", p=128))
                nc.sync.dma_start(dvec_sb[:], dvecc[:])
                nc.sync.dma_start(bs2c_sb[:], bs2c[:])
                nc.sync.dma_start(bdc_sb[:], bdc[:])

                # h1T = relu(W1.T @ x + b1)        [H, N]
                for hc in range(KH):
                    ps = p1ps.tile([128, N], f32, tag="mlp")
                    for kc in range(KD):
                        nc.tensor.matmul(
                            ps[:],
                            _cast(W1_sb[:, kc, hc * 128:(hc + 1) * 128]),
                            _cast(xTb_sb[:, kc]),
                            start=(kc == 0), stop=(kc == KD - 1),
                        )
                    nc.scalar.activation(h1T[:, hc], ps[:], AF.Relu,
                                         bias=b1c_sb[:, hc:hc + 1])

                # h2T = relu(W2.T @ h1 + b2)       [H2, N]
                for mc in range(KH2):
                    ps = p1ps.tile([128, N], f32, tag="mlp")
                    for kc in range(KH):
                        nc.tensor.matmul(
                            ps[:],
                            _cast(W2_sb[:, kc, mc * 128:(mc + 1) * 128]),
                            _cast(h1T[:, kc]),
                            start=(kc == 0), stop=(kc == KH - 1),
                        )
                    nc.scalar.activation(h2T[:, mc], ps[:], AF.Relu,
                                         bias=b2c_sb[:, mc:mc + 1])

                # logits / softmax / loss1 parts, per 128-row tile of the batch
                for mt in range(N // 128):
                    ps6 = p1ps2.tile([128, C], f32, tag="log")
                    for kc in range(KH2):
                        nc.tensor.matmul(
                            ps6[:],
                            _cast(h2T[:, kc, mt * 128:(mt + 1) * 128]),
                            _cast(W3_sb[:, kc]),
                            start=(kc == 0), stop=(kc == KH2 - 1),
                        )
                    l_sb = p1.tile([128, C], f32, tag="lsb")
                    nc.vector.tensor_tensor(l_sb[:], ps6[:], b3bc_sb[:], ALU.add)
                    negmx = p1.tile([128, 1], f32, tag="negmx")
                    nc.vector.tensor_reduce(negmx[:], l_sb[:], axis=AX.X,
                                            op=ALU.max, negate=True)
                    e_sb = p1.tile([128, C], f32, tag="esb")
                    S = p1.tile([128, 1], f32, tag="S")
                    nc.scalar.activation(e_sb[:], l_sb[:], AF.Exp,
                                         bias=negmx[:], accum_out=S[:])
                    rS = p1.tile([128, 1], f32, tag="rS")
                    nc.vector.reciprocal(rS[:], S[:])
                    nc.vector.tensor_scalar(hatp_sb[:, mt], e_sb[:], rS[:], None,
                                            op0=ALU.mult)
                    # loss1 part = l.y - max - ln(S)
                    scr6 = p1.tile([128, C], f32, tag="scr6")
                    ly = p1.tile([128, 1], f32, tag="ly")
                    nc.vector.tensor_tensor(scr6[:], l_sb[:], yfb_sb[:, mt],
                                            ALU.mult)
                    nc.vector.tensor_reduce(ly[:], scr6[:], axis=AX.X,
                                            op=ALU.add)
                    lnS = p1.tile([128, 1], f32, tag="lnS")
                    nc.scalar.activation(lnS[:], S[:], AF.Ln)
                    nc.vector.tensor_tensor(ly[:], ly[:], negmx[:], ALU.add)
                    nc.vector.tensor_tensor(lparts[:, mt:mt + 1], ly[:], lnS[:],
                                            ALU.subtract)

                # a' = Ws1a.T @ x_ours + bs1       [H, RPC]
                for hc in range(KH):
                    ps = p1ps2.tile([128, RPC], f32, tag="aps")
                    for kc in range(KD):
                        nc.tensor.matmul(
                            ps[:],
                            _cast(Ws1a_sb[:, kc, hc * 128:(hc + 1) * 128]),
                            _cast(xTo_sb[:, kc]),
                            start=(kc == 0), stop=(kc == KD - 1),
                        )
                    nc.scalar.activation(aprime[:, hc], ps[:], AF.Identity,
                                         bias=bs1c_sb[:, hc:hc + 1])

                # bT = Ws1b.T @ x_batch            [H, N]
                for hc in range(KH):
                    ps = p1ps.tile([128, N], f32, tag="mlp")
                    for kc in range(KD):
                        nc.tensor.matmul(
                            ps[:],
                            _cast(Ws1b_sb[:, kc, hc * 128:(hc + 1) * 128]),
                            _cast(xTb_sb[:, kc]),
                            start=(kc == 0), stop=(kc == KD - 1),
                        )
                    nc.scalar.activation(bT[:, hc], ps[:], AF.Copy)

                # same matrix rows, aligned with dstack layout
                for l, yside in ((0, yfte_sb), (1, yfto_sb)):
                    psS = p1ps2.tile([NIT, N], f32, tag="same")
                    nc.tensor.matmul(psS[:], yside[:], yftb_sb[:],
                                     start=True, stop=True)
                    nc.scalar.activation(sstack[:, l * N:(l + 1) * N], psS[:],
                                         AF.Copy)

            # ---------------- main pairwise loop
            with tc.tile_pool(name="hq", bufs=3) as hqp, \
                 tc.tile_pool(name="hq2", bufs=8) as hq2p, \
                 tc.tile_pool(name="drow", bufs=3) as drp, \
                 tc.tile_pool(name="hq2ps", bufs=5, space="PSUM") as hq2ps, \
                 tc.tile_pool(name="dps", bufs=2, space="PSUM") as dpsp:
                for t in range(NIT):
                    i0 = 2 * t
                    hq = hqp.tile([128, KH, 2 * N], _MMDT, tag="hq")
                    k = 0
                    for hc in range(KH):
                        for l in range(2):
                            dst = hq[:, hc, l * N:(l + 1) * N]
                            bias_ap = aprime[:, hc, i0 + l:i0 + l + 1]
                            if k % 8 < 5:   # 10 of 16 on ACT, 6 on DVE
                                nc.scalar.activation(dst, bT[:, hc], AF.Relu,
                                                     bias=bias_ap)
                            else:
                                nc.vector.tensor_scalar(
                                    dst, bT[:, hc], bias_ap, 0.0,
                                    op0=ALU.add, op1=ALU.max)
                            k += 1

                    hq2s = []
                    for m in range(KH2):
                        ps = hq2ps.tile([128, 2 * N], f32, tag="hq2ps")
                        for hc in range(KH):
                            nc.tensor.matmul(
                                ps[:],
                                _cast(Ws2_sb[:, hc, m * 128:(m + 1) * 128]),
                                _cast(hq[:, hc]),
                                start=(hc == 0), stop=(hc == KH - 1),
                            )
                        hq2 = hq2p.tile([128, 2 * N], _MMDT, tag="hq2")
                        nc.vector.tensor_scalar(hq2[:], ps[:],
                                                bs2c_sb[:, m:m + 1], 0.0,
                                                op0=ALU.add, op1=ALU.max)
                        hq2s.append(hq2)

                    dps = dpsp.tile([1, 2 * N], f32, tag="dps")
                    for m in range(KH2):
                        nc.tensor.matmul(dps[:], _cast(dvec_sb[:, m:m + 1]),
                                         _cast(hq2s[m][:]),
                                         start=(m == 0), stop=(m == KH2 - 1))
                    drow = drp.tile([1, 2 * N], f32, tag="drow")
                    nc.vector.tensor_copy(drow[:], dps[:])
                    nc.sync.dma_start(dstack[t:t + 1, :], drow[:])

            # ---------------- epilogue: q1, loss2 sums, tilde_p
            with tc.tile_pool(name="end", bufs=1) as ep, \
                 tc.tile_pool(name="endps", bufs=2, space="PSUM") as eps, \
                 tc.tile_pool(name="tps", bufs=1, space="PSUM") as tpsp:
                # delta' = delta + (bs3[1]-bs3[0])
                nc.vector.tensor_scalar(dstack[:], dstack[:], bdc_sb[:], None,
                                        op0=ALU.add)
                q1s = ep.tile([NIT, 2 * N], f32)
                nc.scalar.activation(q1s[:], dstack[:], AF.Sigmoid)
                # softplus sum (Softplus table unavailable in sim: use ln(1+exp))
                scr = ep.tile([NIT, 2 * N], f32)
                nc.scalar.activation(scr[:], dstack[:], AF.Exp)
                nc.scalar.activation(scr[:], scr[:], AF.Ln, bias=1.0,
                                     accum_out=lparts[0:NIT, 2:3])
                scr2 = ep.tile([NIT, 2 * N], f32)
                nc.vector.tensor_tensor(scr2[:], sstack[:], dstack[:], ALU.mult)
                nc.vector.tensor_reduce(lparts[0:NIT, 3:4], scr2[:], axis=AX.X,
                                        op=ALU.add)

                # transpose q1 rows -> q1T[p, jc, t, l] = q1(i=2t+l, j=jc*128+p)
                q1T = ep.tile([128, N // 128, NIT, 2], f32)
                for l in range(2):
                    for jc in range(N // 128):
                        trp = eps.tile([128, NIT], f32, tag="trp")
                        nc.tensor.transpose(
                            trp[:], q1s[:, l * N + jc * 128: l * N + (jc + 1) * 128],
                            ident[:])
                        nc.vector.tensor_copy(q1T[:, jc, :, l], trp[:])

                # tilde = normalize_c( sum_j q1[i,j] hat_p[j,:] )
                tps = tpsp.tile([RPC, C], f32)
                for jc in range(N // 128):
                    nc.tensor.matmul(tps[:], q1T[:, jc], hatp_sb[:, jc],
                                     start=(jc == 0), stop=(jc == N // 128 - 1))
                s6 = ep.tile([RPC, 1], f32)
                nc.vector.tensor_reduce(s6[:], tps[:], axis=AX.X, op=ALU.add)
                rs6 = ep.tile([RPC, 1], f32)
                nc.vector.reciprocal(rs6[:], s6[:])
                tilde_sb = ep.tile([RPC, C], f32)
                nc.vector.tensor_scalar(tilde_sb[:], tps[:], rs6[:], None,
                                        op0=ALU.mult)

                nc.sync.dma_start(hatp_o.rearrange("(mt p) c -> p mt c", p=128),
                                  hatp_sb[:])
                nc.sync.dma_start(tilde_o[:], tilde_sb[:])
                nc.sync.dma_start(lparts_o[:], lparts[:])

    return nc


# ------------------------------------------------------------- host wrapper
_NC_CACHE = None


def _get_program():
    global _NC_CACHE
    if _NC_CACHE is None:
        _NC_CACHE = build_program()
    return _NC_CACHE


def _mm_np(a):
    a = np.asarray(a, dtype=np.float32)
    if MM_MODE == "bf16":
        import ml_dtypes
        return a.astype(ml_dtypes.bfloat16)
    return a


def make_in_maps(agg_r, y, W1, b1, W2, b2, W3, b3, Ws1, bs1, Ws2, bs2, Ws3, bs3):
    agg_r = np.asarray(agg_r, dtype=np.float32)
    yf = np.asarray(y).astype(np.float32)
    f = lambda a: np.ascontiguousarray(np.asarray(a, dtype=np.float32))
    W1, b1, W2, b2, W3, b3 = f(W1), f(b1), f(W2), f(b2), f(W3), f(b3)
    Ws1, bs1, Ws2, bs2, Ws3, bs3 = f(Ws1), f(bs1), f(Ws2), f(bs2), f(Ws3), f(bs3)

    colsplit = lambda v: np.ascontiguousarray(v.reshape(-1, 128).T)
    shared = {
        "W1": _mm_np(W1), "W2": _mm_np(W2), "W3": _mm_np(W3),
        "b1c": colsplit(b1), "b2c": colsplit(b2),
        "b3bc": np.ascontiguousarray(np.tile(b3, (128, 1))),
        "Ws1a": _mm_np(np.ascontiguousarray(Ws1[:D])),
        "Ws1b": _mm_np(np.ascontiguousarray(Ws1[D:])),
        "bs1c": colsplit(bs1),
        "Ws2m": _mm_np(Ws2),
        "bs2c": colsplit(bs2),
        "dvecc": _mm_np(colsplit(Ws3[:, 1] - Ws3[:, 0])),
        "bdc": np.full((NIT, 1), bs3[1] - bs3[0], dtype=np.float32),
    }
    in_maps = []
    for core in range(NCORES):
        b = core // (NCORES // B)
        i0 = (core % (NCORES // B)) * RPC
        xb = agg_r[b]                                   # [N, D]
        yb = np.ascontiguousarray(yf[b])                # [N, C]
        yft = np.ascontiguousarray(yb.T)                # [C, N]
        in_maps.append(dict(
            shared,
            xTb=_mm_np(np.ascontiguousarray(xb.T)),
            xTo=_mm_np(np.ascontiguousarray(xb[i0:i0 + RPC].T)),
            yfb=yb,
            yft_e=np.ascontiguousarray(yft[:, i0:i0 + RPC:2]),
            yft_o=np.ascontiguousarray(yft[:, i0 + 1:i0 + RPC:2]),
            yft_b=yft,
        ))
    return in_maps


def postprocess(results, y):
    hat_p = np.concatenate(
        [results[0]["hatp"], results[NCORES // B]["hatp"]], axis=0
    ).reshape(B, N, C)
    tilde_p = np.concatenate(
        [results[k]["tilde"] for k in range(NCORES)], axis=0
    ).reshape(B, N, C)
    loss1 = -(results[0]["lparts"][:, 0:2].sum(dtype=np.float64)
              + results[NCORES // B]["lparts"][:, 0:2].sum(dtype=np.float64)) / BN
    sp = sum(r["lparts"][:, 2].sum(dtype=np.float64) for r in results)
    sd = sum(r["lparts"][:, 3].sum(dtype=np.float64) for r in results)
    loss2 = (sp - sd) / (B * N * N)
    loss = np.float32(loss1 + loss2)
    seg_y = np.argmax(np.asarray(y), axis=-1).astype(np.int32)
    return loss, hat_p, tilde_p, seg_y


def kernel(agg_r, y, W1, b1, W2, b2, W3, b3, Ws1, bs1, Ws2, bs2, Ws3, bs3,
           **run_kwargs):
    nc = _get_program()
    in_maps = make_in_maps(agg_r, y, W1, b1, W2, b2, W3, b3,
                           Ws1, bs1, Ws2, bs2, Ws3, bs3)
    res = run_bass_kernel_spmd(nc, in_maps, list(range(NCORES)), **run_kwargs)
    out = postprocess(res.results, y)
    kernel.last_run = res
    return out


# revision 6
# speedup vs baseline: 1.0955x; 1.0955x over previous
"""Trainium2 Bass kernel for nn_CLPredictor (dense pairwise MLP).

Computation (see reference):
  per-segment MLP  : hat_p = softmax(MLP3(agg_r)), loss1 = -mean(logp[y])
  pairwise MLP     : for all (i, j) pairs per batch,
                     hq  = relu(a_i + b_j + bs1)         (a/b = agg_r @ Ws1 halves)
                     hq2 = relu(hq @ Ws2 + bs2)
                     delta = hq2 . (Ws3[:,1]-Ws3[:,0]) + (bs3[1]-bs3[0])
                     q1 = sigmoid(delta), loss2 = mean(softplus(delta) - same*delta)
  tilde_p[i]       = normalize(sum_j q1[i,j] * hat_p[j])

Sharding: 8 cores, each owns 64 consecutive (b, i) rows of the B*N x N
pairwise grid (cores 0-3 -> batch 0, cores 4-7 -> batch 1); params
replicated; per-segment MLP computed per-batch (replicated within the 4
cores of a batch; hat_p halves taken from cores 0 and 4).

All activations are kept transposed ([feature, segment]) so the pairwise
broadcast add runs as one ACT/DVE op per tile with a per-partition bias.
"""

import numpy as np

import concourse.bass as bass
import concourse.mybir as mybir
import concourse.tile as tile
from concourse.bass_utils import run_bass_kernel_spmd
from concourse.masks import make_identity

# ---------------------------------------------------------------- constants
B, N, D, H, C = 2, 256, 512, 1024, 6
H2 = H // 2          # 512
BN = B * N           # 512
NCORES = 8
RPC = N // (NCORES // B)   # 64 i-rows per core
NIT = RPC // 2             # 32 main-loop iterations (2 i's each)

f32 = mybir.dt.float32
f32r = mybir.dt.float32r
bf16 = mybir.dt.bfloat16
AF = mybir.ActivationFunctionType
ALU = mybir.AluOpType
AX = mybir.AxisListType

# Matmul mode for the heavy pairwise path and the prologue matmuls.
#   "f32r": fp32 data, fast PE mode (1 cyc/row when N>=256)
#   "bf16": bf16 operands (half SBUF traffic), fp32 PSUM accumulation
#   "f32" : plain fp32 (4 cyc/row)
MM_MODE = "bf16"

# dtype used for every tensor consumed by the fast matmuls (walrus requires
# producers of fp32r matmul operands to emit rounded fp32r, so the tiles and
# DRAM tensors themselves carry the dtype; numpy side stays float32).
_MMDT = {"f32r": f32r, "bf16": bf16, "f32": f32}[MM_MODE]


def _cast(ap):
    return ap


# ------------------------------------------------- walrus wait workaround
# This container's walrus accepts at most ONE sync wait per instruction
# (setupSyncWait: "Too many sync wait commands"), while Tile freely
# attaches several.  Two patches:
#  1. after wait assignment, split any instruction with >1 waits by
#     hoisting the extras onto same-engine InstNoOp carriers emitted just
#     before it (engine program order then gates the instruction);
#  2. the Tile exit Drain (which waits on the whole global clock) gets the
#     same treatment via nop carriers on the sync engine.
_MAX_WAITS = 1


def _patch_tile_waits():
    import bass_rust as _br
    from concourse.vector_clock import ScopedClock

    orig_lower = tile.TileContext._lower_ordered_insts

    def _split_lower(self, ordered):
        nc = self.nc
        for bb_name, insts in ordered.items():
            new_insts = []
            for inst in insts:
                si = getattr(inst, "sync_info", None)
                waits = list(si.on_wait) if si is not None and si.on_wait else []
                if len(waits) > _MAX_WAITS and not isinstance(
                    inst, tile.BassTileLoopBlock
                ):
                    keep = waits[-_MAX_WAITS:]
                    extra = waits[:-_MAX_WAITS]
                    for w in extra:
                        nop = mybir.InstNoOp(
                            name=f"I-{nc.next_id()}",
                            sync_info=mybir.SyncInfo(on_wait=[w], on_update=[]),
                            bass_nofuse=True,
                            engine=inst.engine,
                        )
                        new_insts.append(nop)
                    si.on_wait = keep
                new_insts.append(inst)
            insts[:] = new_insts
        return orig_lower(self, ordered)

    tile.TileContext._lower_ordered_insts = _split_lower

    def _split_drain_and_barrier(self, tick_clock, wait_clock):
        nc = self.nc
        carrier = nc.sync.nop()
        wait_clock.add_sem_waits(
            carrier.ins, ScopedClock({None: tick_clock.global_clock})
        )
        si = carrier.ins.sync_info
        waits = list(si.on_wait) if si and si.on_wait else []
        if len(waits) > 1:
            si.on_wait = waits[:1]
            for w in waits[1:]:
                extra = nc.sync.nop()
                extra.ins.sync_info = _br.SyncInfo(on_wait=[w], on_update=[])

        nc.sync.drain()
        nc.all_engine_barrier()
        assert self.sems is not None
        popped = nc._tile_sem_poison_stack.pop()
        assert popped is self._sem_poison
        nc.clear_and_free_semaphores(list(self.sems.allocated().values()))
        nc.all_engine_barrier()

    tile.TileContext._drain_and_barrier = _split_drain_and_barrier


_patch_tile_waits()


# ------------------------------------------------------------ device program
def build_program():
    nc = bass.Bass("TRN2", target_bir_lowering=False, debug=False)

    def din(name, shape, dt=f32):
        return nc.dram_tensor(name, shape, dt, kind="ExternalInput")

    # per-core inputs (b = core's batch, i0 = core's first local row)
    xTb = din("xTb", [D, N], _MMDT)   # agg_r[b].T
    xTo = din("xTo", [D, RPC], _MMDT) # agg_r[b][i0:i0+RPC].T
    yfb = din("yfb", [N, C])          # one-hot y[b] as f32
    yft_e = din("yft_e", [C, NIT])    # yfb.T cols i0, i0+2, ...
    yft_o = din("yft_o", [C, NIT])    # yfb.T cols i0+1, i0+3, ...
    yft_b = din("yft_b", [C, N])      # yfb.T
    # replicated params
    W1 = din("W1", [D, H], _MMDT)
    W2 = din("W2", [H, H2], _MMDT)
    W3 = din("W3", [H2, C], _MMDT)
    b1c = din("b1c", [128, H // 128])
    b2c = din("b2c", [128, H2 // 128])
    b3bc = din("b3bc", [128, C])
    Ws1a = din("Ws1a", [D, H], _MMDT)
    Ws1b = din("Ws1b", [D, H], _MMDT)
    bs1c = din("bs1c", [128, H // 128])
    Ws2m = din("Ws2m", [H, H2], _MMDT)
    bs2c = din("bs2c", [128, H2 // 128])
    dvecc = din("dvecc", [128, H2 // 128], _MMDT)
    bdc = din("bdc", [NIT, 1])

    hatp_o = nc.dram_tensor("hatp", [N, C], f32, kind="ExternalOutput")
    tilde_o = nc.dram_tensor("tilde", [RPC, C], f32, kind="ExternalOutput")
    lparts_o = nc.dram_tensor("lparts", [128, 6], f32, kind="ExternalOutput")

    KD = D // 128    # 4 contraction chunks over D
    KH = H // 128    # 8 chunks over H
    KH2 = H2 // 128  # 4 chunks over H2

    with tile.TileContext(nc) as tc:
        # ---------------- persistent tiles (live across the whole kernel)
        with tc.tile_pool(name="persist", bufs=1) as pp:
            bT = pp.tile([128, KH, N], f32)          # b-side, batch cols
            aprime = pp.tile([128, KH, RPC], f32)    # a-side + bs1, our cols
            Ws2_sb = pp.tile([128, KH, H2], _MMDT)
            dvec_sb = pp.tile([128, KH2], _MMDT)
            bs2c_sb = pp.tile([128, KH2], f32)
            bdc_sb = pp.tile([NIT, 1], f32)
            hatp_sb = pp.tile([128, N // 128, C], f32)
            dstack = pp.tile([NIT, 2 * N], f32)      # delta rows
            sstack = pp.tile([NIT, 2 * N], f32)      # same rows
            lparts = pp.tile([128, 6], f32)
            ident = pp.tile([NIT, NIT], f32)

            make_identity(nc, ident[:])
            nc.vector.memset(lparts[:], 0.0)

            # ---------------- phase 1: per-seg MLP + a/b precompute
            with tc.tile_pool(name="ph1", bufs=1) as p1, \
                 tc.tile_pool(name="ph1ps", bufs=2, space="PSUM") as p1ps, \
                 tc.tile_pool(name="ph1ps2", bufs=2, space="PSUM") as p1ps2:
                xTb_sb = p1.tile([128, KD, N], _MMDT)
                xTo_sb = p1.tile([128, KD, RPC], _MMDT)
                W1_sb = p1.tile([128, KD, H], _MMDT)
                W2_sb = p1.tile([128, KH, H2], _MMDT)
                W3_sb = p1.tile([128, KH2, C], _MMDT)
                Ws1a_sb = p1.tile([128, KD, H], _MMDT)
                Ws1b_sb = p1.tile([128, KD, H], _MMDT)
                b1c_sb = p1.tile([128, KH], f32)
                b2c_sb = p1.tile([128, KH2], f32)
                b3bc_sb = p1.tile([128, C], f32)
                bs1c_sb = p1.tile([128, KH], f32)
                yfb_sb = p1.tile([128, N // 128, C], f32)
                yfte_sb = p1.tile([C, NIT], f32)
                yfto_sb = p1.tile([C, NIT], f32)
                yftb_sb = p1.tile([C, N], f32)
                h1T = p1.tile([128, KH, N], _MMDT)
                h2T = p1.tile([128, KH2, N], _MMDT)

                nc.sync.dma_start(xTb_sb[:], xTb.rearrange("(kc p) m -> p kc m", p=128))
                nc.sync.dma_start(xTo_sb[:], xTo.rearrange("(kc p) m -> p kc m", p=128))
                nc.sync.dma_start(W1_sb[:], W1.rearrange("(kc p) m -> p kc m", p=128))
                nc.sync.dma_start(W2_sb[:], W2.rearrange("(kc p) m -> p kc m", p=128))
                nc.sync.dma_start(W3_sb[:], W3.rearrange("(kc p) m -> p kc m", p=128))
                nc.sync.dma_start(Ws1a_sb[:], Ws1a.rearrange("(kc p) m -> p kc m", p=128))
                nc.sync.dma_start(Ws1b_sb[:], Ws1b.rearrange("(kc p) m -> p kc m", p=128))
                nc.sync.dma_start(b1c_sb[:], b1c[:])
                nc.sync.dma_start(b2c_sb[:], b2c[:])
                nc.sync.dma_start(b3bc_sb[:], b3bc[:])
                nc.sync.dma_start(bs1c_sb[:], bs1c[:])
                nc.sync.dma_start(yfb_sb[:], yfb.rearrange("(mt p) c -> p mt c", p=128))
                nc.sync.dma_start(yfte_sb[:], yft_e[:])
                nc.sync.dma_start(yfto_sb[:], yft_o[:])
                nc.sync.dma_start(yftb_sb[:], yft_b[:])

                # persist-pool loads issued after the phase-1 weights so the
                # first h1 matmuls aren't stuck behind the 2MB Ws2 transfer
                nc.sync.dma_start(Ws2_sb[:], Ws2m.rearrange("(kc p) m -> p kc m", p=128))
                nc.sync.dma_start(dvec_sb[:], dvecc[:])
                nc.sync.dma_start(bs2c_sb[:], bs2c[:])
                nc.sync.dma_start(bdc_sb[:], bdc[:])

                # h1T = relu(W1.T @ x + b1)        [H, N]
                for hc in range(KH):
                    ps = p1ps.tile([128, N], f32, tag="mlp")
                    for kc in range(KD):
                        nc.tensor.matmul(
                            ps[:],
                            _cast(W1_sb[:, kc, hc * 128:(hc + 1) * 128]),
                            _cast(xTb_sb[:, kc]),
                            start=(kc == 0), stop=(kc == KD - 1),
                        )
                    nc.scalar.activation(h1T[:, hc], ps[:], AF.Relu,
                                         bias=b1c_sb[:, hc:hc + 1])

                # h2T = relu(W2.T @ h1 + b2)       [H2, N]
                for mc in range(KH2):
                    ps = p1ps.tile([128, N], f32, tag="mlp")
                    for kc in range(KH):
                        nc.tensor.matmul(
                            ps[:],
                            _cast(W2_sb[:, kc, mc * 128:(mc + 1) * 128]),
                            _cast(h1T[:, kc]),
                            start=(kc == 0), stop=(kc == KH - 1),
                        )
                    nc.scalar.activation(h2T[:, mc], ps[:], AF.Relu,
                                         bias=b2c_sb[:, mc:mc + 1])

                # logits / softmax / loss1 parts, per 128-row tile of the batch
                for mt in range(N // 128):
                    ps6 = p1ps2.tile([128, C], f32, tag="log")
                    for kc in range(KH2):
                        nc.tensor.matmul(
                            ps6[:],
                            _cast(h2T[:, kc, mt * 128:(mt + 1) * 128]),
                            _cast(W3_sb[:, kc]),
                            start=(kc == 0), stop=(kc == KH2 - 1),
                        )
                    l_sb = p1.tile([128, C], f32, tag="lsb")
                    nc.vector.tensor_tensor(l_sb[:], ps6[:], b3bc_sb[:], ALU.add)
                    negmx = p1.tile([128, 1], f32, tag="negmx")
                    nc.vector.tensor_reduce(negmx[:], l_sb[:], axis=AX.X,
                                            op=ALU.max, negate=True)
                    e_sb = p1.tile([128, C], f32, tag="esb")
                    S = p1.tile([128, 1], f32, tag="S")
                    nc.scalar.activation(e_sb[:], l_sb[:], AF.Exp,
                                         bias=negmx[:], accum_out=S[:])
                    rS = p1.tile([128, 1], f32, tag="rS")
                    nc.vector.reciprocal(rS[:], S[:])
                    nc.vector.tensor_scalar(hatp_sb[:, mt], e_sb[:], rS[:], None,
                                            op0=ALU.mult)
                    # loss1 part = l.y - max - ln(S)
                    scr6 = p1.tile([128, C], f32, tag="scr6")
                    ly = p1.tile([128, 1], f32, tag="ly")
                    nc.vector.tensor_tensor(scr6[:], l_sb[:], yfb_sb[:, mt],
                                            ALU.mult)
                    nc.vector.tensor_reduce(ly[:], scr6[:], axis=AX.X,
                                            op=ALU.add)
                    lnS = p1.tile([128, 1], f32, tag="lnS")
                    nc.scalar.activation(lnS[:], S[:], AF.Ln)
                    nc.vector.tensor_tensor(ly[:], ly[:], negmx[:], ALU.add)
                    nc.vector.tensor_tensor(lparts[:, mt:mt + 1], ly[:], lnS[:],
                                            ALU.subtract)

                # a' = Ws1a.T @ x_ours + bs1       [H, RPC]
                for hc in range(KH):
                    ps = p1ps2.tile([128, RPC], f32, tag="aps")
                    for kc in range(KD):
                        nc.tensor.matmul(
                            ps[:],
                            _cast(Ws1a_sb[:, kc, hc * 128:(hc + 1) * 128]),
                            _cast(xTo_sb[:, kc]),
                            start=(kc == 0), stop=(kc == KD - 1),
                        )
                    nc.scalar.activation(aprime[:, hc], ps[:], AF.Identity,
                                         bias=bs1c_sb[:, hc:hc + 1])

                # bT = Ws1b.T @ x_batch            [H, N]
                for hc in range(KH):
                    ps = p1ps.tile([128, N], f32, tag="mlp")
                    for kc in range(KD):
                        nc.tensor.matmul(
                            ps[:],
                            _cast(Ws1b_sb[:, kc, hc * 128:(hc + 1) * 128]),
                            _cast(xTb_sb[:, kc]),
                            start=(kc == 0), stop=(kc == KD - 1),
                        )
                    nc.scalar.activation(bT[:, hc], ps[:], AF.Copy)

                # same matrix rows, aligned with dstack layout
                for l, yside in ((0, yfte_sb), (1, yfto_sb)):
                    psS = p1ps2.tile([NIT, N], f32, tag="same")
                    nc.tensor.matmul(psS[:], yside[:], yftb_sb[:],
                                     start=True, stop=True)
                    nc.scalar.activation(sstack[:, l * N:(l + 1) * N], psS[:],
                                         AF.Copy)

            # ---------------- main pairwise loop
            with tc.tile_pool(name="hq", bufs=3) as hqp, \
                 tc.tile_pool(name="hq2", bufs=8) as hq2p, \
                 tc.tile_pool(name="drow", bufs=3) as drp, \
                 tc.tile_pool(name="hq2ps", bufs=5, space="PSUM") as hq2ps, \
                 tc.tile_pool(name="dps", bufs=2, space="PSUM") as dpsp:
                for t in range(NIT):
                    i0 = 2 * t
                    hq = hqp.tile([128, KH, 2 * N], _MMDT, tag="hq")
                    k = 0
                    for hc in range(KH):
                        for l in range(2):
                            dst = hq[:, hc, l * N:(l + 1) * N]
                            bias_ap = aprime[:, hc, i0 + l:i0 + l + 1]
                            if k % 8 < 5:   # 10 of 16 on ACT, 6 on DVE
                                nc.scalar.activation(dst, bT[:, hc], AF.Relu,
                                                     bias=bias_ap)
                            else:
                                nc.vector.tensor_scalar(
                                    dst, bT[:, hc], bias_ap, 0.0,
                                    op0=ALU.add, op1=ALU.max)
                            k += 1

                    hq2s = []
                    for m in range(KH2):
                        ps = hq2ps.tile([128, 2 * N], f32, tag="hq2ps")
                        for hc in range(KH):
                            nc.tensor.matmul(
                                ps[:],
                                _cast(Ws2_sb[:, hc, m * 128:(m + 1) * 128]),
                                _cast(hq[:, hc]),
                                start=(hc == 0), stop=(hc == KH - 1),
                            )
                        hq2 = hq2p.tile([128, 2 * N], _MMDT, tag="hq2")
                        nc.vector.tensor_scalar(hq2[:], ps[:],
                                                bs2c_sb[:, m:m + 1], 0.0,
                                                op0=ALU.add, op1=ALU.max)
                        hq2s.append(hq2)

                    dps = dpsp.tile([1, 2 * N], f32, tag="dps")
                    for m in range(KH2):
                        nc.tensor.matmul(dps[:], _cast(dvec_sb[:, m:m + 1]),
                                         _cast(hq2s[m][:]),
                                         start=(m == 0), stop=(m == KH2 - 1))
                    drow = drp.tile([1, 2 * N], f32, tag="drow")
                    nc.vector.tensor_copy(drow[:], dps[:])
                    nc.sync.dma_start(dstack[t:t + 1, :], drow[:])

            # ---------------- epilogue: q1, loss2 sums, tilde_p
            with tc.tile_pool(name="end", bufs=1) as ep, \
                 tc.tile_pool(name="endps", bufs=2, space="PSUM") as eps, \
                 tc.tile_pool(name="tps", bufs=1, space="PSUM") as tpsp:
                # delta' = delta + (bs3[1]-bs3[0])
                nc.vector.tensor_scalar(dstack[:], dstack[:], bdc_sb[:], None,
                                        op0=ALU.add)
                q1s = ep.tile([NIT, 2 * N], f32)
                nc.scalar.activation(q1s[:], dstack[:], AF.Sigmoid)
                # softplus sum (Softplus table unavailable in sim: use ln(1+exp))
                scr = ep.tile([NIT, 2 * N], f32)
                nc.scalar.activation(scr[:], dstack[:], AF.Exp)
                nc.scalar.activation(scr[:], scr[:], AF.Ln, bias=1.0,
                                     accum_out=lparts[0:NIT, 2:3])
                scr2 = ep.tile([NIT, 2 * N], f32)
                nc.vector.tensor_tensor(scr2[:], sstack[:], dstack[:], ALU.mult)
                nc.vector.tensor_reduce(lparts[0:NIT, 3:4], scr2[:], axis=AX.X,
                                        op=ALU.add)

                # transpose q1 rows -> q1T[p, jc, t, l] = q1(i=2t+l, j=jc*128+p)
                q1T = ep.tile([128, N // 128, NIT, 2], f32)
                for l in range(2):
                    for jc in range(N // 128):
                        trp = eps.tile([128, NIT], f32, tag="trp")
                        nc.tensor.transpose(
                            trp[:], q1s[:, l * N + jc * 128: l * N + (jc + 1) * 128],
                            ident[:])
                        nc.vector.tensor_copy(q1T[:, jc, :, l], trp[:])

                # tilde = normalize_c( sum_j q1[i,j] hat_p[j,:] )
                tps = tpsp.tile([RPC, C], f32)
                for jc in range(N // 128):
                    nc.tensor.matmul(tps[:], q1T[:, jc], hatp_sb[:, jc],
                                     start=(jc == 0), stop=(jc == N // 128 - 1))
                s6 = ep.tile([RPC, 1], f32)
                nc.vector.tensor_reduce(s6[:], tps[:], axis=AX.X, op=ALU.add)
                rs6 = ep.tile([RPC, 1], f32)
                nc.vector.reciprocal(rs6[:], s6[:])
                tilde_sb = ep.tile([RPC, C], f32)
                nc.vector.tensor_scalar(tilde_sb[:], tps[:], rs6[:], None,
                                        op0=ALU.mult)

                nc.sync.dma_start(hatp_o.rearrange("(mt p) c -> p mt c", p=128),
                                  hatp_sb[:])
                nc.sync.dma_start(tilde_o[:], tilde_sb[:])
                nc.sync.dma_start(lparts_o[:], lparts[:])

    return nc


# ------------------------------------------------------------- host wrapper
_NC_CACHE = None


def _get_program():
    global _NC_CACHE
    if _NC_CACHE is None:
        _NC_CACHE = build_program()
    return _NC_CACHE


def _mm_np(a):
    a = np.asarray(a, dtype=np.float32)
    if MM_MODE == "bf16":
        import ml_dtypes
        return a.astype(ml_dtypes.bfloat16)
    return a


def make_in_maps(agg_r, y, W1, b1, W2, b2, W3, b3, Ws1, bs1, Ws2, bs2, Ws3, bs3):
    agg_r = np.asarray(agg_r, dtype=np.float32)
    yf = np.asarray(y).astype(np.float32)
    f = lambda a: np.ascontiguousarray(np.asarray(a, dtype=np.float32))
    W1, b1, W2, b2, W3, b3 = f(W1), f(b1), f(W2), f(b2), f(W3), f(b3)
    Ws1, bs1, Ws2, bs2, Ws3, bs3 = f(Ws1), f(bs1), f(Ws2), f(bs2), f(Ws3), f(bs3)

    colsplit = lambda v: np.ascontiguousarray(v.reshape(-1, 128).T)
    shared = {
        "W1": _mm_np(W1), "W2": _mm_np(W2), "W3": _mm_np(W3),
        "b1c": colsplit(b1), "b2c": colsplit(b2),
        "b3bc": np.ascontiguousarray(np.tile(b3, (128, 1))),
        "Ws1a": _mm_np(np.ascontiguousarray(Ws1[:D])),
        "Ws1b": _mm_np(np.ascontiguousarray(Ws1[D:])),
        "bs1c": colsplit(bs1),
        "Ws2m": _mm_np(Ws2),
        "bs2c": colsplit(bs2),
        "dvecc": _mm_np(colsplit(Ws3[:, 1] - Ws3[:, 0])),
        "bdc": np.full((NIT, 1), bs3[1] - bs3[0], dtype=np.float32),
    }
    in_maps = []
    for core in range(NCORES):
        b = core // (NCORES // B)
        i0 = (core % (NCORES // B)) * RPC
        xb = agg_r[b]                                   # [N, D]
        yb = np.ascontiguousarray(yf[b])                # [N, C]
        yft = np.ascontiguousarray(yb.T)                # [C, N]
        in_maps.append(dict(
            shared,
            xTb=_mm_np(np.ascontiguousarray(xb.T)),
            xTo=_mm_np(np.ascontiguousarray(xb[i0:i0 + RPC].T)),
            yfb=yb,
            yft_e=np.ascontiguousarray(yft[:, i0:i0 + RPC:2]),
            yft_o=np.ascontiguousarray(yft[:, i0 + 1:i0 + RPC:2]),
            yft_b=yft,
        ))
    return in_maps


def postprocess(results, y):
    hat_p = np.concatenate(
        [results[0]["hatp"], results[NCORES // B]["hatp"]], axis=0
    ).reshape(B, N, C)
    tilde_p = np.concatenate(
        [results[k]["tilde"] for k in range(NCORES)], axis=0
    ).reshape(B, N, C)
    loss1 = -(results[0]["lparts"][:, 0:2].sum(dtype=np.float64)
              + results[NCORES // B]["lparts"][:, 0:2].sum(dtype=np.float64)) / BN
    sp = sum(r["lparts"][:, 2].sum(dtype=np.float64) for r in results)
    sd = sum(r["lparts"][:, 3].sum(dtype=np.float64) for r in results)
    loss2 = (sp - sd) / (B * N * N)
    loss = np.float32(loss1 + loss2)
    seg_y = np.argmax(np.asarray(y), axis=-1).astype(np.int32)
    return loss, hat_p, tilde_p, seg_y


def kernel(agg_r, y, W1, b1, W2, b2, W3, b3, Ws1, bs1, Ws2, bs2, Ws3, bs3,
           **run_kwargs):
    nc = _get_program()
    in_maps = make_in_maps(agg_r, y, W1, b1, W2, b2, W3, b3,
                           Ws1, bs1, Ws2, bs2, Ws3, bs3)
    res = run_bass_kernel_spmd(nc, in_maps, list(range(NCORES)), **run_kwargs)
    out = postprocess(res.results, y)
    kernel.last_run = res
    return out
